# revision 26
# baseline (speedup 1.0000x reference)
"""MixtureOfSoftMaxACF Trainium2 kernel (fp16, ACT-bound pipeline).

Per-core (data-parallel over BS=8 across 8 cores, batch b per core):
  qt[b] memory reinterpreted as QQ[2, 2048, 64] (contiguous halves), same kt.
  For m in {0,1}:  S_m = QQ[m] @ KK[m].T / sqrt(128);  P_m = softmax(S_m, axis=-1)
  out[b] = (p0 * P_0 + p1 * P_1) @ vt[b]
  p: mixture prior (softmax over batch axis) -> computed on host, passed per-core.

Device pipeline per core, per (qh, m):
  - Scores: S^T [128 keys, 1024 q] = lhsT(K^T chunk [64,128]) @ rhs(Q^T slab),
    fp16 inputs, two 512-wide matmuls (PSUM bank limit).
  - exp on ScalarE from PSUM -> E fp16 SBUF (scale=1/sqrt(128)); ScalarE is the
    bottleneck engine, everything else is arranged to hide behind it.
  - AV (V-stationary) deferred one chunk so PE never waits on the current exp.
  - Softmax denominator OFF the PE: DVE pairwise adds E(2j)+E(2j+1) in-loop
    (fp16 2x mode), small tree post-loop, then one (ones/p_m)-stationary matmul
    replicates D/p_m across partitions; reciprocal gives p_m/D directly.
  - Normalize in the [dv, q] domain, sum mixtures, PE-transpose to [q, dv],
    DVE copy, DMA out. Cross-boundary work overlaps the next chunk loop.
"""

import math
from contextlib import ExitStack

import numpy as np

import concourse.bass as bass
import concourse.bacc as bacc
import concourse.mybir as mybir
import concourse.tile as tile
from concourse.bass_utils import run_bass_kernel_spmd
from concourse.masks import make_identity

BS = 8
N = 2048          # queries
NK = 2048         # keys
DK = 128
M = 2
D = DK // M       # 64
DV = 128
TEMP = math.sqrt(DK)
NCH = NK // 128   # 16 key chunks
QH = 2            # query halves
QHN = N // QH     # 1024

F32 = mybir.dt.float32
F16 = mybir.dt.float16

_NC = None
LAST_RESULT = None  # BassKernelResults of last run (test.py reads this)


def _build():
    nc = bacc.Bacc(None)
    qt_d = nc.declare_dram_parameter("qt_b", [DK, N], F16, isOutput=False)
    kt_d = nc.declare_dram_parameter("kt_b", [DK, NK], F16, isOutput=False)
    vt_d = nc.declare_dram_parameter("vt_b", [NK, DK], F16, isOutput=False)
    pr_d = nc.declare_dram_parameter("pr_b", [1, M], F32, isOutput=False)
    out_d = nc.declare_dram_parameter("out_b", [N, DK], F32, isOutput=True)

    with ExitStack() as ctx:
        tc = ctx.enter_context(tile.TileContext(nc))
        const = ctx.enter_context(tc.tile_pool(name="const", bufs=1))
        sbig = ctx.enter_context(tc.tile_pool(name="sbig", bufs=1))
        epool = ctx.enter_context(tc.tile_pool(name="epool", bufs=8))
        eppool = ctx.enter_context(tc.tile_pool(name="eppool", bufs=2))
        dpool = ctx.enter_context(tc.tile_pool(name="dpool", bufs=2))
        npool = ctx.enter_context(tc.tile_pool(name="npool", bufs=2))
        opool = ctx.enter_context(tc.tile_pool(name="opool", bufs=1))
        ps_s = ctx.enter_context(tc.tile_pool(name="ps_s", bufs=3, space="PSUM"))
        ps_acc = ctx.enter_context(tc.tile_pool(name="ps_acc", bufs=1, space="PSUM"))

        # ---- identity first: it gates every PE transpose and is built on
        # gpsimd, whose queue also issues DMAs below
        ident_f = const.tile([128, 128], F32)
        make_identity(nc, ident_f)
        ones_f = const.tile([128, 128], F32)
        nc.vector.memset(ones_f, 1.0)
        warm_w = const.tile([128, 128], F16, tag="warm_w")
        nc.vector.memset(warm_w, 1.0)
        warm_r = const.tile([128, 512], F16, tag="warm_r")
        nc.vector.memset(warm_r, 0.5)

        # ---- staging: inputs arrive fp16 and pre-transposed from the host
        # (qt/kt as [128 dk, n]), so staging is pure contiguous DMA -- no PE
        # transposes, no dtype casts. Split so the first chunks unblock early.
        qt_t = sbig.tile([128, N], F16)
        kt_t = sbig.tile([128, NK], F16)
        v_sb = sbig.tile([128, NCH, DV], F16)
        pr_sb = const.tile([128, M], F32)
        nc.gpsimd.dma_start(
            out=pr_sb,
            in_=bass.AP(tensor=pr_d, offset=0, ap=[[0, 128], [1, M]]),
        )
        for h in range(2):
            nc.sync.dma_start(
                out=qt_t[:, h * 1024:(h + 1) * 1024],
                in_=bass.AP(tensor=qt_d, offset=h * 1024,
                            ap=[[N, 128], [1, 1024]]),
            )
        for g in range(4):
            nc.scalar.dma_start(
                out=kt_t[:, g * 512:(g + 1) * 512],
                in_=bass.AP(tensor=kt_d, offset=g * 512,
                            ap=[[NK, 128], [1, 512]]),
            )
        for h in range(2):
            nc.gpsimd.dma_start(
                out=v_sb[:, h * 8:(h + 1) * 8, :],
                in_=bass.AP(tensor=vt_d, offset=h * 8 * 128 * DK,
                            ap=[[DK, 128], [128 * DK, 8], [1, DV]]),
            )

        warm_ps = ps_acc.tile([128, QHN], F32, tag="outT")

        def warmup(n):
            # dependency-free matmuls bridge the fixed preamble + DMA wait so
            # the HAM clock gate is warming before the first real matmul
            for _ in range(n):
                nc.tensor.matmul(warm_ps[:, 0:512], lhsT=warm_w, rhs=warm_r,
                                 start=True, stop=True)

        warmup(7)

        pr_rec = const.tile([128, M], F32)
        ones_p = []
        for m in range(M):
            t = const.tile([128, 128], F16, tag=f"ones_p{m}")
            ones_p.append(t)

        def prior_setup():
            # 1/p_m broadcast on all partitions -> ones/p_m weight tiles
            nc.vector.reciprocal(pr_rec, pr_sb)
            for m in range(M):
                nc.vector.tensor_scalar_mul(ones_p[m], ones_f,
                                            pr_rec[:, m:m + 1])

        # ---- phase 2+3: attention ----
        # Normalization of segment i is deferred into segment i+1's chunk loop
        # (tree-adds early, Drep matmul late) so the PE queue never blocks on
        # DVE work at a boundary: PE goes straight from AV(15) to next S(0),
        # which also keeps the HAM clock-gate warm.
        scale = 1.0 / TEMP
        stage1 = {}  # c -> closure, emitted during the chunk loop
        stage2 = []  # closures, emitted after the full sweep

        def emit_stage1(c):
            for fn in stage1.pop(c, ()):  # noqa: B909
                fn()

        otn = {}
        daccs = {}
        ocs = {}
        drecs = {}
        rAs = {}
        rT2s = {}
        stage1[3] = [prior_setup]
        segs = [(qh, m) for qh in range(QH) for m in range(M)]
        for qh, m in segs:
            is_last = (qh, m) == segs[-1]
            seg0 = (qh, m) == segs[0]
            av_done = 0
            outT = ps_acc.tile([128, QHN], F32, tag="outT")
            ep = eppool.tile([128, NCH // 2, QHN], F16, tag="ep")
            es = []
            racc = None
            for c in range(NCH):
                s = ps_s.tile([128, QHN], F32, tag="s")
                for hf in range(2):
                    nc.tensor.matmul(
                        s[:, hf * 512:(hf + 1) * 512],
                        lhsT=kt_t[m * D:(m + 1) * D, c * 128:(c + 1) * 128],
                        rhs=qt_t[m * D:(m + 1) * D,
                                 qh * QHN + hf * 512: qh * QHN + (hf + 1) * 512],
                        start=True, stop=True,
                    )
                # deferred AV: consume already-exp'd chunks so the PE never
                # waits on the ScalarE. Segment 0 defers harder (DMA/cold-PE
                # startup): 0 pairs before chunk 6, then catch up.
                if seg0:
                    quota = 0 if c < 6 else (1 if c % 2 == 0 else 2)
                    if c < 6:
                        warmup(1)
                else:
                    quota = 2 if c >= 2 else 0
                while quota > 0 and av_done < min(c - 1, NCH - 2):
                    for hf in range(2):
                        sl = slice(hf * 512, (hf + 1) * 512)
                        nc.tensor.matmul(outT[:, sl], lhsT=v_sb[:, av_done, :],
                                         rhs=es[av_done][:, sl],
                                         start=(av_done == 0), stop=False)
                    av_done += 1
                    quota -= 1
                E = epool.tile([128, QHN], F16, tag="E")
                nc.scalar.activation(E, s, mybir.ActivationFunctionType.Exp,
                                     scale=scale)
                es.append(E)
                if c % 2 == 1 and not (is_last and c >= 12):
                    nc.vector.tensor_add(ep[:, c // 2, :], es[c - 1], es[c])
                # last segment: running sum so Dacc is ready one DVE op after
                # the final exp (pairs up to chunk 11, then singles)
                if is_last:
                    addend = None
                    if c % 2 == 1 and 3 <= c <= 11:
                        addend = ep[:, c // 2, :] if c > 3 else None
                        if c == 3:
                            r = dpool.tile([128, QHN], F16, tag="racc")
                            nc.vector.tensor_add(r, ep[:, 0, :], ep[:, 1, :])
                            racc = r
                            addend = None
                    elif c >= 12:
                        addend = es[c]
                    if addend is not None:
                        r = dpool.tile([128, QHN], F16, tag="racc")
                        nc.vector.tensor_add(r, racc, addend)
                        racc = r
                emit_stage1(c)
            # tail AVs (pending chunks) + outT copy-out: for non-final
            # segments these are deferred into the next segment's first two
            # chunks so its score matmuls reach the ScalarE with no gap.
            def tail_avs(outT=outT, es=es, lst=tuple(range(av_done, NCH - 1))):
                for ct in lst:
                    for hf in range(2):
                        sl = slice(hf * 512, (hf + 1) * 512)
                        nc.tensor.matmul(outT[:, sl], lhsT=v_sb[:, ct, :],
                                         rhs=es[ct][:, sl],
                                         start=False, stop=False)

            def tail_last(outT=outT, es=es, qh=qh, m=m, is_last=is_last):
                for hf in range(2):
                    sl = slice(hf * 512, (hf + 1) * 512)
                    nc.tensor.matmul(outT[:, sl], lhsT=v_sb[:, NCH - 1, :],
                                     rhs=es[NCH - 1][:, sl],
                                     start=False, stop=True)
                if not is_last:
                    oc = opool.tile([128, QHN], F32, tag=f"oc{qh}{m}")
                    nc.vector.tensor_copy(oc, outT)
                    ocs[(qh, m)] = oc

            if is_last:
                tail_avs()
                tail_last()

            def tree(qh=qh, m=m, ep=ep):
                # denominator tree: 8 pairs -> 4 -> 2 -> 1 (fp16), on DVE
                t4 = dpool.tile([128, 4, QHN], F16, tag="t4")
                nc.vector.tensor_add(t4, ep[:, 0:4, :], ep[:, 4:8, :])
                t2 = dpool.tile([128, 2, QHN], F16, tag="t2")
                nc.vector.tensor_add(t2, t4[:, 0:2, :], t4[:, 2:4, :])
                dacc = dpool.tile([128, QHN], F16, tag=f"dacc{m}")
                nc.vector.tensor_add(dacc, t2[:, 0, :], t2[:, 1, :])
                daccs[(qh, m)] = dacc

            def norm(qh=qh, m=m):
                # replicate D/p_m across partitions with a ones/p_m matmul
                dacc = daccs.pop((qh, m))
                Drep = ps_s.tile([128, QHN], F32, tag="s")
                for hf in range(2):
                    sl = slice(hf * 512, (hf + 1) * 512)
                    nc.tensor.matmul(Drep[:, sl], lhsT=ones_p[m], rhs=dacc[:, sl],
                                     start=True, stop=True)
                drec = opool.tile([128, QHN], F32, tag=f"drec{qh}{m}")
                nc.vector.reciprocal_approx_fast(drec, Drep)
                drecs[(qh, m)] = drec

            def finish_mul(qh=qh):
                # DVE part, scheduled two chunks before the transposes so the
                # PE never queues behind an unfinished DVE chain
                rA = npool.tile([128, QHN], F32, tag="rA")
                nc.vector.tensor_mul(rA, ocs[(qh, 0)], drecs[(qh, 0)])
                rB = npool.tile([128, QHN], F32, tag="rB")
                nc.vector.tensor_mul(rB, ocs[(qh, 1)], drecs[(qh, 1)])
                rT2 = npool.tile([128, QHN], F32, tag="rT2")
                nc.vector.tensor_add(rT2, rA, rB)
                rT2s[qh] = rT2

            def finish_t(qh=qh):
                rT2 = rT2s.pop(qh)
                res_ps = ps_s.tile([128, QHN], F32, tag="s")
                for t in range(QHN // 128):
                    nc.tensor.transpose(res_ps[:, t * 128:(t + 1) * 128],
                                        rT2[:, t * 128:(t + 1) * 128], ident_f)
                res_sb = npool.tile([128, QHN], F32, tag="res")
                nc.vector.tensor_copy(res_sb, res_ps)
                nc.sync.dma_start(
                    out=bass.AP(tensor=out_d, offset=qh * QHN * DK,
                                ap=[[DK, 128], [128 * DK, QHN // 128], [1, DV]]),
                    in_=res_sb.rearrange("p (t d) -> p t d", d=DV),
                )

            def finishA(qh=qh):
                rA = npool.tile([128, QHN], F32, tag="rA")
                nc.vector.tensor_mul(rA, ocs[(qh, 0)], drecs[(qh, 0)])
                rAs[qh] = rA

            def finishB(qh=qh):
                rB = npool.tile([128, QHN], F32, tag="rB")
                nc.vector.tensor_mul(rB, ocs[(qh, 1)], drecs[(qh, 1)])
                rT2 = npool.tile([128, QHN], F32, tag="rT2")
                nc.vector.tensor_add(rT2, rAs[qh], rB)
                res_ps = ps_s.tile([128, QHN], F32, tag="s")
                for t in range(QHN // 128):
                    nc.tensor.transpose(res_ps[:, t * 128:(t + 1) * 128],
                                        rT2[:, t * 128:(t + 1) * 128], ident_f)
                res_sb = npool.tile([128, QHN], F32, tag="res")
                nc.vector.tensor_copy(res_sb, res_ps)
                nc.sync.dma_start(
                    out=bass.AP(tensor=out_d, offset=qh * QHN * DK,
                                ap=[[DK, 128], [128 * DK, QHN // 128], [1, DV]]),
                    in_=res_sb.rearrange("p (t d) -> p t d", d=DV),
                )

            if not is_last:
                stage1.setdefault(0, []).append(tail_avs)
                stage1.setdefault(1, []).append(tail_last)
                stage1.setdefault(2, []).append(tree)
                stage1.setdefault(8, []).append(norm)
                if m == M - 1:
                    stage1.setdefault(11, []).append(finish_mul)
                    stage1.setdefault(13, []).append(finish_t)
                if (qh, m) == segs[-2]:
                    # rA of the final qh can be computed as soon as its m=0
                    # drec lands, during the last segment's loop
                    stage1.setdefault(10, []).append(finishA)
            else:
                # halfwise tail: Drep/drec/mul/add then transpose/copy/DMA per
                # 512-half so DVE, PE and DMA overlap instead of serializing
                dacc = racc
                Drep = ps_s.tile([128, QHN], F32, tag="s")
                drec = opool.tile([128, QHN], F32, tag=f"drec{qh}{m}")
                rB = npool.tile([128, QHN], F32, tag="rB")
                rT2 = npool.tile([128, QHN], F32, tag="rT2")
                for hf in range(2):
                    sl = slice(hf * 512, (hf + 1) * 512)
                    nc.tensor.matmul(Drep[:, sl], lhsT=ones_p[m], rhs=dacc[:, sl],
                                     start=True, stop=True)
                    nc.vector.reciprocal_approx_fast(drec[:, sl], Drep[:, sl])
                    nc.vector.tensor_mul(rB[:, sl], outT[:, sl], drec[:, sl])
                    nc.vector.tensor_add(rT2[:, sl], rAs[qh][:, sl], rB[:, sl])
                res_ps = ps_s.tile([128, QHN], F32, tag="s")
                res_sb = npool.tile([128, QHN], F32, tag="res")
                for hf in range(2):
                    sl = slice(hf * 512, (hf + 1) * 512)
                    for t in range(4):
                        tt = hf * 4 + t
                        nc.tensor.transpose(res_ps[:, tt * 128:(tt + 1) * 128],
                                            rT2[:, tt * 128:(tt + 1) * 128],
                                            ident_f)
                    nc.vector.tensor_copy(res_sb[:, sl], res_ps[:, sl])
                    nc.sync.dma_start(
                        out=bass.AP(
                            tensor=out_d,
                            offset=qh * QHN * DK + hf * 4 * 128 * DK,
                            ap=[[DK, 128], [128 * DK, 4], [1, DV]]),
                        in_=res_sb[:, sl].rearrange("p (t d) -> p t d", d=DV),
                    )
        stage1.clear()
    return nc


def _get_nc():
    global _NC
    if _NC is None:
        _NC = _build()
        _NC.finalize()  # Bacc.compile(): event sems, reg alloc, wait legalization
    return _NC


def _prior(qt, kernel):
    bar_qt = qt.astype(np.float32).mean(axis=1)          # (BS, dk)
    logits = kernel.astype(np.float32) @ bar_qt.T        # (m, BS)
    z = logits - logits.max(axis=1, keepdims=True)
    ez = np.exp(z)
    pm = ez / ez.sum(axis=1, keepdims=True)              # softmax over batch axis
    return pm.reshape(-1)


def kernel(qt, kt, vt, kernel):
    global LAST_RESULT
    import os
    nc = _get_nc()
    prior_flat = _prior(qt, kernel)
    in_maps = []
    for b in range(BS):
        pr = np.array([[prior_flat[2 * b], prior_flat[2 * b + 1]]], dtype=np.float32)
        in_maps.append({
            # replicate the reference's row-major [N,128]->[M,N,64] mixture
            # reshape, then lay out d-major: row m*64+d, col n
            "qt_b": np.ascontiguousarray(
                qt[b].astype(np.float16).reshape(M, N, D)
                .transpose(0, 2, 1).reshape(DK, N)),
            "kt_b": np.ascontiguousarray(
                kt[b].astype(np.float16).reshape(M, NK, D)
                .transpose(0, 2, 1).reshape(DK, NK)),
            "vt_b": np.ascontiguousarray(vt[b], dtype=np.float16),
            "pr_b": pr,
        })
    trace = bool(int(os.environ.get("KERNEL_TRACE", "0")))
    res = run_bass_kernel_spmd(nc, in_maps, list(range(BS)), trace=trace)
    LAST_RESULT = res
    out = np.stack([np.asarray(res.results[b]["out_b"]).reshape(N, DK) for b in range(BS)])
    return out.astype(np.float32)


# revision 27
# speedup vs baseline: 1.1101x; 1.1101x over previous
"""MixtureOfSoftMaxACF Trainium2 kernel (fp16, ACT-bound pipeline).

Per-core (data-parallel over BS=8 across 8 cores, batch b per core):
  qt[b] memory reinterpreted as QQ[2, 2048, 64] (contiguous halves), same kt.
  For m in {0,1}:  S_m = QQ[m] @ KK[m].T / sqrt(128);  P_m = softmax(S_m, axis=-1)
  out[b] = (p0 * P_0 + p1 * P_1) @ vt[b]
  p: mixture prior (softmax over batch axis) -> computed on host, passed per-core.

Device pipeline per core, per (qh, m):
  - Scores: S^T [128 keys, 1024 q] = lhsT(K^T chunk [64,128]) @ rhs(Q^T slab),
    fp16 inputs, two 512-wide matmuls (PSUM bank limit).
  - exp on ScalarE from PSUM -> E fp16 SBUF (scale=1/sqrt(128)); ScalarE is the
    bottleneck engine, everything else is arranged to hide behind it.
  - AV (V-stationary) deferred one chunk so PE never waits on the current exp.
  - Softmax denominator OFF the PE: DVE pairwise adds E(2j)+E(2j+1) in-loop
    (fp16 2x mode), small tree post-loop, then one (ones/p_m)-stationary matmul
    replicates D/p_m across partitions; reciprocal gives p_m/D directly.
  - Normalize in the [dv, q] domain, sum mixtures, PE-transpose to [q, dv],
    DVE copy, DMA out. Cross-boundary work overlaps the next chunk loop.
"""

import math
from contextlib import ExitStack

import numpy as np

import concourse.bass as bass
import concourse.bacc as bacc
import concourse.mybir as mybir
import concourse.tile as tile
from concourse.bass_utils import run_bass_kernel_spmd
from concourse.masks import make_identity

BS = 8
N = 2048          # queries
NK = 2048         # keys
DK = 128
M = 2
D = DK // M       # 64
DV = 128
TEMP = math.sqrt(DK)
NCH = NK // 128   # 16 key chunks
QH = 2            # query halves
QHN = N // QH     # 1024

F32 = mybir.dt.float32
F16 = mybir.dt.float16

_NC = None
LAST_RESULT = None  # BassKernelResults of last run (test.py reads this)


def _build():
    nc = bacc.Bacc(None)
    qt_d = nc.declare_dram_parameter("qt_b", [DK, N], F16, isOutput=False)
    kt_d = nc.declare_dram_parameter("kt_b", [DK, NK], F16, isOutput=False)
    vt_d = nc.declare_dram_parameter("vt_b", [NK, DK], F16, isOutput=False)
    pr_d = nc.declare_dram_parameter("pr_b", [1, M], F32, isOutput=False)
    out_d = nc.declare_dram_parameter("out_b", [N, DK], F32, isOutput=True)

    with ExitStack() as ctx:
        tc = ctx.enter_context(tile.TileContext(nc))
        const = ctx.enter_context(tc.tile_pool(name="const", bufs=1))
        sbig = ctx.enter_context(tc.tile_pool(name="sbig", bufs=1))
        epool = ctx.enter_context(tc.tile_pool(name="epool", bufs=8))
        eppool = ctx.enter_context(tc.tile_pool(name="eppool", bufs=2))
        dpool = ctx.enter_context(tc.tile_pool(name="dpool", bufs=2))
        npool = ctx.enter_context(tc.tile_pool(name="npool", bufs=2))
        opool = ctx.enter_context(tc.tile_pool(name="opool", bufs=1))
        ps_s = ctx.enter_context(tc.tile_pool(name="ps_s", bufs=3, space="PSUM"))
        ps_acc = ctx.enter_context(tc.tile_pool(name="ps_acc", bufs=1, space="PSUM"))

        # ---- identity first: it gates every PE transpose and is built on
        # gpsimd, whose queue also issues DMAs below
        ident_f = const.tile([128, 128], F32)
        make_identity(nc, ident_f)
        ones_f = const.tile([128, 128], F32)
        nc.vector.memset(ones_f, 1.0)
        warm_w = const.tile([128, 128], F16, tag="warm_w")
        nc.vector.memset(warm_w, 1.0)
        warm_r = const.tile([128, 512], F16, tag="warm_r")
        nc.vector.memset(warm_r, 0.5)

        # ---- staging: inputs arrive fp16 and pre-transposed from the host
        # (qt/kt as [128 dk, n]), so staging is pure contiguous DMA -- no PE
        # transposes, no dtype casts. Split so the first chunks unblock early.
        qt_t = sbig.tile([128, N], F16)
        kt_t = sbig.tile([128, NK], F16)
        v_sb = sbig.tile([128, NCH, DV], F16)
        pr_sb = const.tile([128, M], F32)
        nc.gpsimd.dma_start(
            out=pr_sb,
            in_=bass.AP(tensor=pr_d, offset=0, ap=[[0, 128], [1, M]]),
        )
        for h in range(2):
            nc.sync.dma_start(
                out=qt_t[:, h * 1024:(h + 1) * 1024],
                in_=bass.AP(tensor=qt_d, offset=h * 1024,
                            ap=[[N, 128], [1, 1024]]),
            )
        for g in range(4):
            nc.scalar.dma_start(
                out=kt_t[:, g * 512:(g + 1) * 512],
                in_=bass.AP(tensor=kt_d, offset=g * 512,
                            ap=[[NK, 128], [1, 512]]),
            )
        for h in range(2):
            nc.gpsimd.dma_start(
                out=v_sb[:, h * 8:(h + 1) * 8, :],
                in_=bass.AP(tensor=vt_d, offset=h * 8 * 128 * DK,
                            ap=[[DK, 128], [128 * DK, 8], [1, DV]]),
            )

        warm_ps = ps_acc.tile([128, QHN], F32, tag="outT")

        def warmup(n):
            # dependency-free matmuls bridge the fixed preamble + DMA wait so
            # the HAM clock gate is warming before the first real matmul
            for _ in range(n):
                nc.tensor.matmul(warm_ps[:, 0:512], lhsT=warm_w, rhs=warm_r,
                                 start=True, stop=True)

        warmup(12)

        pr_rec = const.tile([128, M], F32)
        ones_p = []
        for m in range(M):
            t = const.tile([128, 128], F16, tag=f"ones_p{m}")
            ones_p.append(t)

        def prior_setup():
            # 1/p_m broadcast on all partitions -> ones/p_m weight tiles
            nc.vector.reciprocal(pr_rec, pr_sb)
            for m in range(M):
                nc.vector.tensor_scalar_mul(ones_p[m], ones_f,
                                            pr_rec[:, m:m + 1])

        # ---- phase 2+3: attention ----
        # Normalization of segment i is deferred into segment i+1's chunk loop
        # (tree-adds early, Drep matmul late) so the PE queue never blocks on
        # DVE work at a boundary: PE goes straight from AV(15) to next S(0),
        # which also keeps the HAM clock-gate warm.
        scale = 1.0 / TEMP
        stage1 = {}  # c -> closure, emitted during the chunk loop
        stage2 = []  # closures, emitted after the full sweep

        def emit_stage1(c):
            for fn in stage1.pop(c, ()):  # noqa: B909
                fn()

        otn = {}
        daccs = {}
        ocs = {}
        drecs = {}
        rAs = {}
        rT2s = {}
        stage1[3] = [prior_setup]
        segs = [(qh, m) for qh in range(QH) for m in range(M)]
        for qh, m in segs:
            is_last = (qh, m) == segs[-1]
            seg0 = (qh, m) == segs[0]
            av_done = 0
            outT = ps_acc.tile([128, QHN], F32, tag="outT")
            ep = eppool.tile([128, NCH // 2, QHN], F16, tag="ep")
            es = []
            racc = None
            for c in range(NCH):
                s = ps_s.tile([128, QHN], F32, tag="s")
                for hf in range(2):
                    nc.tensor.matmul(
                        s[:, hf * 512:(hf + 1) * 512],
                        lhsT=kt_t[m * D:(m + 1) * D, c * 128:(c + 1) * 128],
                        rhs=qt_t[m * D:(m + 1) * D,
                                 qh * QHN + hf * 512: qh * QHN + (hf + 1) * 512],
                        start=True, stop=True,
                    )
                # deferred AV: consume already-exp'd chunks so the PE never
                # waits on the ScalarE. Segment 0 defers harder (DMA/cold-PE
                # startup): 0 pairs before chunk 6, then catch up.
                if seg0:
                    quota = 0 if c < 6 else (1 if c % 2 == 0 else 2)
                    if c < 6:
                        warmup(1)
                else:
                    quota = 2 if c >= 2 else 0
                while quota > 0 and av_done < min(c - 1, NCH - 2):
                    for hf in range(2):
                        sl = slice(hf * 512, (hf + 1) * 512)
                        nc.tensor.matmul(outT[:, sl], lhsT=v_sb[:, av_done, :],
                                         rhs=es[av_done][:, sl],
                                         start=(av_done == 0), stop=False)
                    av_done += 1
                    quota -= 1
                E = epool.tile([128, QHN], F16, tag="E")
                nc.scalar.activation(E, s, mybir.ActivationFunctionType.Exp,
                                     scale=scale)
                es.append(E)
                if c % 2 == 1 and not (is_last and c >= 12):
                    nc.vector.tensor_add(ep[:, c // 2, :], es[c - 1], es[c])
                # last segment: running sum so Dacc is ready one DVE op after
                # the final exp (pairs up to chunk 11, then singles)
                if is_last:
                    addend = None
                    if c % 2 == 1 and 3 <= c <= 11:
                        addend = ep[:, c // 2, :] if c > 3 else None
                        if c == 3:
                            r = dpool.tile([128, QHN], F16, tag="racc")
                            nc.vector.tensor_add(r, ep[:, 0, :], ep[:, 1, :])
                            racc = r
                            addend = None
                    elif c >= 12:
                        addend = es[c]
                    if addend is not None:
                        r = dpool.tile([128, QHN], F16, tag="racc")
                        nc.vector.tensor_add(r, racc, addend)
                        racc = r
                emit_stage1(c)
            # tail AVs (pending chunks) + outT copy-out: for non-final
            # segments these are deferred into the next segment's first two
            # chunks so its score matmuls reach the ScalarE with no gap.
            def tail_avs(outT=outT, es=es, lst=tuple(range(av_done, NCH - 1))):
                for ct in lst:
                    for hf in range(2):
                        sl = slice(hf * 512, (hf + 1) * 512)
                        nc.tensor.matmul(outT[:, sl], lhsT=v_sb[:, ct, :],
                                         rhs=es[ct][:, sl],
                                         start=False, stop=False)

            def tail_last(outT=outT, es=es, qh=qh, m=m, is_last=is_last):
                for hf in range(2):
                    sl = slice(hf * 512, (hf + 1) * 512)
                    nc.tensor.matmul(outT[:, sl], lhsT=v_sb[:, NCH - 1, :],
                                     rhs=es[NCH - 1][:, sl],
                                     start=False, stop=True)
                if not is_last:
                    oc = opool.tile([128, QHN], F32, tag=f"oc{qh}{m}")
                    nc.vector.tensor_copy(oc, outT)
                    ocs[(qh, m)] = oc

            if is_last:
                tail_avs()
                tail_last()

            def tree(qh=qh, m=m, ep=ep):
                # denominator tree: 8 pairs -> 4 -> 2 -> 1 (fp16), on DVE
                t4 = dpool.tile([128, 4, QHN], F16, tag="t4")
                nc.vector.tensor_add(t4, ep[:, 0:4, :], ep[:, 4:8, :])
                t2 = dpool.tile([128, 2, QHN], F16, tag="t2")
                nc.vector.tensor_add(t2, t4[:, 0:2, :], t4[:, 2:4, :])
                dacc = dpool.tile([128, QHN], F16, tag=f"dacc{m}")
                nc.vector.tensor_add(dacc, t2[:, 0, :], t2[:, 1, :])
                daccs[(qh, m)] = dacc

            def norm(qh=qh, m=m):
                # replicate D/p_m across partitions with a ones/p_m matmul
                dacc = daccs.pop((qh, m))
                Drep = ps_s.tile([128, QHN], F32, tag="s")
                for hf in range(2):
                    sl = slice(hf * 512, (hf + 1) * 512)
                    nc.tensor.matmul(Drep[:, sl], lhsT=ones_p[m], rhs=dacc[:, sl],
                                     start=True, stop=True)
                drec = opool.tile([128, QHN], F32, tag=f"drec{qh}{m}")
                nc.vector.reciprocal_approx_fast(drec, Drep)
                drecs[(qh, m)] = drec

            def finish_mul(qh=qh):
                # DVE part, scheduled two chunks before the transposes so the
                # PE never queues behind an unfinished DVE chain
                rA = npool.tile([128, QHN], F32, tag="rA")
                nc.vector.tensor_mul(rA, ocs[(qh, 0)], drecs[(qh, 0)])
                rB = npool.tile([128, QHN], F32, tag="rB")
                nc.vector.tensor_mul(rB, ocs[(qh, 1)], drecs[(qh, 1)])
                rT2 = npool.tile([128, QHN], F32, tag="rT2")
                nc.vector.tensor_add(rT2, rA, rB)
                rT2s[qh] = rT2

            def finish_t(qh=qh):
                rT2 = rT2s.pop(qh)
                res_ps = ps_s.tile([128, QHN], F32, tag="s")
                for t in range(QHN // 128):
                    nc.tensor.transpose(res_ps[:, t * 128:(t + 1) * 128],
                                        rT2[:, t * 128:(t + 1) * 128], ident_f)
                res_sb = npool.tile([128, QHN], F32, tag="res")
                nc.vector.tensor_copy(res_sb, res_ps)
                nc.sync.dma_start(
                    out=bass.AP(tensor=out_d, offset=qh * QHN * DK,
                                ap=[[DK, 128], [128 * DK, QHN // 128], [1, DV]]),
                    in_=res_sb.rearrange("p (t d) -> p t d", d=DV),
                )

            def finishA(qh=qh):
                rA = npool.tile([128, QHN], F32, tag="rA")
                nc.vector.tensor_mul(rA, ocs[(qh, 0)], drecs[(qh, 0)])
                rAs[qh] = rA

            def finishB(qh=qh):
                rB = npool.tile([128, QHN], F32, tag="rB")
                nc.vector.tensor_mul(rB, ocs[(qh, 1)], drecs[(qh, 1)])
                rT2 = npool.tile([128, QHN], F32, tag="rT2")
                nc.vector.tensor_add(rT2, rAs[qh], rB)
                res_ps = ps_s.tile([128, QHN], F32, tag="s")
                for t in range(QHN // 128):
                    nc.tensor.transpose(res_ps[:, t * 128:(t + 1) * 128],
                                        rT2[:, t * 128:(t + 1) * 128], ident_f)
                res_sb = npool.tile([128, QHN], F32, tag="res")
                nc.vector.tensor_copy(res_sb, res_ps)
                nc.sync.dma_start(
                    out=bass.AP(tensor=out_d, offset=qh * QHN * DK,
                                ap=[[DK, 128], [128 * DK, QHN // 128], [1, DV]]),
                    in_=res_sb.rearrange("p (t d) -> p t d", d=DV),
                )

            if not is_last:
                stage1.setdefault(0, []).append(tail_avs)
                stage1.setdefault(1, []).append(tail_last)
                stage1.setdefault(2, []).append(tree)
                stage1.setdefault(8, []).append(norm)
                if m == M - 1:
                    stage1.setdefault(11, []).append(finish_mul)
                    stage1.setdefault(13, []).append(finish_t)
                if (qh, m) == segs[-2]:
                    # rA of the final qh can be computed as soon as its m=0
                    # drec lands, during the last segment's loop
                    stage1.setdefault(10, []).append(finishA)
            else:
                # halfwise tail: Drep/drec/mul/add then transpose/copy/DMA per
                # 512-half so DVE, PE and DMA overlap instead of serializing
                dacc = racc
                Drep = ps_s.tile([128, QHN], F32, tag="s")
                drec = opool.tile([128, QHN], F32, tag=f"drec{qh}{m}")
                rB = npool.tile([128, QHN], F32, tag="rB")
                rT2 = npool.tile([128, QHN], F32, tag="rT2")
                for hf in range(2):
                    sl = slice(hf * 512, (hf + 1) * 512)
                    nc.tensor.matmul(Drep[:, sl], lhsT=ones_p[m], rhs=dacc[:, sl],
                                     start=True, stop=True)
                    nc.vector.reciprocal_approx_fast(drec[:, sl], Drep[:, sl])
                    nc.vector.tensor_mul(rB[:, sl], outT[:, sl], drec[:, sl])
                    nc.vector.tensor_add(rT2[:, sl], rAs[qh][:, sl], rB[:, sl])
                res_ps = ps_s.tile([128, QHN], F32, tag="s")
                res_sb = npool.tile([128, QHN], F32, tag="res")
                for hf in range(2):
                    sl = slice(hf * 512, (hf + 1) * 512)
                    for t in range(4):
                        tt = hf * 4 + t
                        nc.tensor.transpose(res_ps[:, tt * 128:(tt + 1) * 128],
                                            rT2[:, tt * 128:(tt + 1) * 128],
                                            ident_f)
                    nc.vector.tensor_copy(res_sb[:, sl], res_ps[:, sl])
                    nc.sync.dma_start(
                        out=bass.AP(
                            tensor=out_d,
                            offset=qh * QHN * DK + hf * 4 * 128 * DK,
                            ap=[[DK, 128], [128 * DK, 4], [1, DV]]),
                        in_=res_sb[:, sl].rearrange("p (t d) -> p t d", d=DV),
                    )
        stage1.clear()
    return nc


def _get_nc():
    global _NC
    if _NC is None:
        _NC = _build()
        _NC.finalize()  # Bacc.compile(): event sems, reg alloc, wait legalization
    return _NC


def _prior(qt, kernel):
    bar_qt = qt.astype(np.float32).mean(axis=1)          # (BS, dk)
    logits = kernel.astype(np.float32) @ bar_qt.T        # (m, BS)
    z = logits - logits.max(axis=1, keepdims=True)
    ez = np.exp(z)
    pm = ez / ez.sum(axis=1, keepdims=True)              # softmax over batch axis
    return pm.reshape(-1)


def kernel(qt, kt, vt, kernel):
    global LAST_RESULT
    import os
    nc = _get_nc()
    prior_flat = _prior(qt, kernel)
    in_maps = []
    for b in range(BS):
        pr = np.array([[prior_flat[2 * b], prior_flat[2 * b + 1]]], dtype=np.float32)
        in_maps.append({
            # replicate the reference's row-major [N,128]->[M,N,64] mixture
            # reshape, then lay out d-major: row m*64+d, col n
            "qt_b": np.ascontiguousarray(
                qt[b].astype(np.float16).reshape(M, N, D)
                .transpose(0, 2, 1).reshape(DK, N)),
            "kt_b": np.ascontiguousarray(
                kt[b].astype(np.float16).reshape(M, NK, D)
                .transpose(0, 2, 1).reshape(DK, NK)),
            "vt_b": np.ascontiguousarray(vt[b], dtype=np.float16),
            "pr_b": pr,
        })
    trace = bool(int(os.environ.get("KERNEL_TRACE", "0")))
    res = run_bass_kernel_spmd(nc, in_maps, list(range(BS)), trace=trace)
    LAST_RESULT = res
    out = np.stack([np.asarray(res.results[b]["out_b"]).reshape(N, DK) for b in range(BS)])
    return out.astype(np.float32)
